# revision 5
# baseline (speedup 1.0000x reference)
"""MoE feed-forward (top-1 routing) on 8 Trainium2 NeuronCores.

Strategy (expert-parallel, per sharding hint):
  - Host computes gate logits only to DECIDE routing (argmax), then
    gathers each expert's tokens and ships them (transposed) to the core
    owning that expert, padded to a common capacity C ("all-to-all" done
    host-side since full inputs arrive on host anyway).
  - Each core re-computes the gate softmax max-probability w for its own
    tokens ON DEVICE (gate replicated), runs the expert FFN
    (x @ W1 -> GLU -> @ W2, fp32r matmuls), scales by w, and emits the
    masked sum of w (for the utilization loss).
  - Host scatters rows back and finishes the 8-element loss formula.
"""

import os
import numpy as np

import concourse.bass as bass
import concourse.bacc as bacc
import concourse.mybir as mybir
import concourse.tile as tile
from concourse.bass_utils import run_bass_kernel_spmd

f32 = mybir.dt.float32
f32r = mybir.dt.float32r
AF = mybir.ActivationFunctionType
ALU = mybir.AluOpType

B, S, D, F, E = 4, 2048, 1024, 2048, 8
T = B * S
P = 128
KD = D // P     # 8 k-tiles for matmul1 / gate
KF = F // P     # 16 k-tiles for matmul2
NI = F // P     # 16 feature-block pairs in stage 1
M2 = D // P     # 8 output m-tiles in stage 2

LAST_RESULTS = None




def _round_fp32r(a):
    """Round fp32 array to fp32r (tf32-like: 11-bit mantissa, RNE)."""
    b = np.ascontiguousarray(a, np.float32).view(np.uint32)
    bias = np.uint32(0x7FF) + ((b >> np.uint32(12)) & np.uint32(1))
    return ((b + bias) & np.uint32(0xFFFFF000)).view(np.float32)


def _chunks(C):
    """Split C into column chunks, each a multiple of 128 in [256, 512]."""
    n = -(-C // 512)
    if C // n < 256:
        n = max(1, C // 256)
    base = C // n
    base -= base % 128
    sizes = [base] * n
    rem = C - base * n
    i = 0
    while rem > 0:
        add = min(128, rem)
        sizes[i % n] += add
        rem -= add
        i += 1
    assert sum(sizes) == C and all(s % 128 == 0 and s <= 512 for s in sizes)
    out, c0 = [], 0
    for s in sizes:
        out.append((c0, s))
        c0 += s
    return out


def _build(C):
    nc = bacc.Bacc()
    chunks = _chunks(C)

    xtr = nc.declare_dram_parameter("xtr", [KD, P, C], f32r, isOutput=False)
    w1r = nc.declare_dram_parameter("w1r", [NI, P, 2 * KD, P], f32r, isOutput=False)
    w2r = nc.declare_dram_parameter("w2r", [M2, P, KF, P], f32r, isOutput=False)
    b1r = nc.declare_dram_parameter("b1r", [P, 2 * NI], f32, isOutput=False)
    b2r = nc.declare_dram_parameter("b2r", [P, M2], f32, isOutput=False)
    gwr = nc.declare_dram_parameter("gwr", [P, KD * E], f32r, isOutput=False)
    gbr = nc.declare_dram_parameter("gbr", [E, 1], f32, isOutput=False)
    mkr = nc.declare_dram_parameter("mkr", [1, C], f32, isOutput=False)
    ytr = nc.declare_dram_parameter("ytr", [M2, P, C], f32, isOutput=True)
    svr = nc.declare_dram_parameter("svr", [1, 1], f32, isOutput=True)

    with tile.TileContext(nc) as tc:
        with (
            tc.tile_pool(name="const", bufs=1) as const,
            tc.tile_pool(name="xt", bufs=1) as xtp,
            tc.tile_pool(name="g", bufs=1) as gp,
            tc.tile_pool(name="w1", bufs=2) as w1p,
            tc.tile_pool(name="w2", bufs=2) as w2p,
            tc.tile_pool(name="ge", bufs=2) as gep,
            tc.tile_pool(name="y", bufs=2) as yp,
            tc.tile_pool(name="rows", bufs=1) as rows,
            tc.tile_pool(name="psmm", bufs=6, space="PSUM") as psmm,
            tc.tile_pool(name="pssm", bufs=2, space="PSUM") as pssm,
        ):
            # ---- constants / small inputs ----
            b1s = const.tile([P, 2 * NI], f32, tag="b1s")
            nc.sync.dma_start(b1s[:], b1r[:])
            b2s = const.tile([P, M2], f32, tag="b2s")
            nc.sync.dma_start(b2s[:], b2r[:])
            gws = const.tile([P, KD * E], f32r, tag="gws")
            nc.sync.dma_start(gws[:], gwr[:])
            gbs = const.tile([E, 1], f32, tag="gbs")
            nc.sync.dma_start(gbs[:], gbr[:])
            masks = const.tile([1, C], f32, tag="masks")
            nc.sync.dma_start(masks[:], mkr[:])
            ones8 = const.tile([E, 1], f32, tag="ones8")
            nc.vector.memset(ones8[:], 1.0)
            ones1 = const.tile([1, P], f32, tag="ones1")
            nc.vector.memset(ones1[:], 1.0)

            # ---- resident token activations xT ----
            xt = []
            for k in range(KD):
                t = xtp.tile([P, C], f32r, tag=f"xt{k}")
                nc.sync.dma_start(t[:], xtr[k])
                xt.append(t)

            # ---- gate: logits^T [8, C] -> w_row [1, C], wb [128, C] ----
            eT = rows.tile([E, C], f32, tag="eT")
            ssum = rows.tile([1, C], f32, tag="ssum")
            for c0, cl in chunks:
                psg = pssm.tile([E, 512], f32, tag="pssmall")
                for k in range(KD):
                    nc.tensor.matmul(
                        psg[:, :cl],
                        gws[:, E * k : E * (k + 1)],
                        xt[k][:, c0 : c0 + cl],
                        start=(k == 0),
                        stop=(k == KD - 1),
                    )
                # e = exp(logits + gate_b)
                nc.scalar.activation(eT[:, c0 : c0 + cl], psg[:, :cl], AF.Exp, bias=gbs[:])
                pss = pssm.tile([1, 512], f32, tag="pssmall")
                nc.tensor.matmul(
                    pss[:, :cl],
                    ones8[:],
                    eT[:, c0 : c0 + cl],
                    start=True,
                    stop=True,
                )
                nc.vector.tensor_copy(ssum[:, c0 : c0 + cl], pss[:, :cl])
            pmax = rows.tile([1, C], f32, tag="pmax")
            nc.gpsimd.tensor_reduce(pmax[:], eT[:], op=ALU.max, axis=mybir.AxisListType.C)
            rin = rows.tile([1, C], f32, tag="rin")
            nc.vector.reciprocal(rin[:], ssum[:])
            w_row = rows.tile([1, C], f32, tag="w_row")
            nc.vector.tensor_mul(w_row[:], pmax[:], rin[:])
            # masked sum of w -> svr
            wm = rows.tile([1, C], f32, tag="wm")
            nc.vector.tensor_mul(wm[:], w_row[:], masks[:])
            sc = rows.tile([1, 1], f32, tag="sc")
            nc.vector.reduce_sum(sc[:], wm[:], axis=mybir.AxisListType.X)
            nc.sync.dma_start(svr[:], sc[:])
            # broadcast w_row across 128 partitions via K=1 matmul
            wb = rows.tile([P, C], f32, tag="wb")
            for c0, cl in chunks:
                psb = pssm.tile([P, 512], f32, tag="pssmall")
                nc.tensor.matmul(
                    psb[:, :cl],
                    ones1[:],
                    w_row[:, c0 : c0 + cl],
                    start=True,
                    stop=True,
                )
                nc.vector.tensor_copy(wb[:, c0 : c0 + cl], psb[:, :cl])

            # ---- stage 1: H^T pairs -> G^T tiles (resident) ----
            g = [gp.tile([P, C], f32r, tag=f"g{i}", name=f"g{i}") for i in range(NI)]
            for i in range(NI):
                w1t = w1p.tile([P, 2 * KD * P], f32r, tag="w1t")
                nc.sync.dma_start(w1t[:], w1r[i])
                for c0, cl in chunks:
                    psa = psmm.tile([P, 512], f32, tag="ps")
                    for k in range(KD):
                        nc.tensor.matmul(
                            psa[:, :cl],
                            w1t[:, P * k : P * (k + 1)],
                            xt[k][:, c0 : c0 + cl],
                            start=(k == 0),
                            stop=(k == KD - 1),
                        )
                    psb2 = psmm.tile([P, 512], f32, tag="ps")
                    for k in range(KD):
                        nc.tensor.matmul(
                            psb2[:, :cl],
                            w1t[:, P * (KD + k) : P * (KD + k + 1)],
                            xt[k][:, c0 : c0 + cl],
                            start=(k == 0),
                            stop=(k == KD - 1),
                        )
                    get = gep.tile([P, 512], f32, tag="get")
                    nc.scalar.activation(
                        get[:, :cl], psb2[:, :cl], AF.Gelu,
                        bias=b1s[:, 2 * i + 1 : 2 * i + 2],
                    )
                    nc.vector.scalar_tensor_tensor(
                        g[i][:, c0 : c0 + cl],
                        psa[:, :cl],
                        b1s[:, 2 * i : 2 * i + 1],
                        get[:, :cl],
                        ALU.add,
                        ALU.mult,
                    )

            # ---- stage 2: Y^T = G^T(k2-tiles) x W2, +b2, *w ----
            for m2 in range(M2):
                w2t = w2p.tile([P, KF * P], f32r, tag="w2t")
                nc.sync.dma_start(w2t[:], w2r[m2])
                for c0, cl in chunks:
                    psy = psmm.tile([P, 512], f32, tag="ps")
                    for k2 in range(KF):
                        nc.tensor.matmul(
                            psy[:, :cl],
                            w2t[:, P * k2 : P * (k2 + 1)],
                            g[k2][:, c0 : c0 + cl],
                            start=(k2 == 0),
                            stop=(k2 == KF - 1),
                        )
                    yt = yp.tile([P, 512], f32, tag="yt")
                    nc.vector.scalar_tensor_tensor(
                        yt[:, :cl],
                        psy[:, :cl],
                        b2s[:, m2 : m2 + 1],
                        wb[:, c0 : c0 + cl],
                        ALU.add,
                        ALU.mult,
                    )
                    nc.sync.dma_start(ytr[m2, :, c0 : c0 + cl], yt[:, :cl])

    nc.finalize()
    return nc


def kernel(x, gate_W, gate_b, fc_W, fc_b, out_W, out_b):
    global LAST_RESULTS
    x = np.ascontiguousarray(np.asarray(x, dtype=np.float32))
    gate_W = np.asarray(gate_W, dtype=np.float32)
    gate_b = np.asarray(gate_b, dtype=np.float32)
    fc_W = np.asarray(fc_W, dtype=np.float32)
    fc_b = np.asarray(fc_b, dtype=np.float32)
    out_W = np.asarray(out_W, dtype=np.float32)
    out_b = np.asarray(out_b, dtype=np.float32)

    xf = x.reshape(T, D)

    # --- routing decision (host): mirror reference softmax/argmax ---
    logits = xf @ gate_W + gate_b
    lm = logits.max(axis=-1, keepdims=True)
    pe = np.exp(logits - lm)
    probs = pe / pe.sum(axis=-1, keepdims=True)
    idx = np.argmax(probs, axis=-1)
    counts = np.bincount(idx, minlength=E).astype(np.int64)

    C = int(max(512, -(-int(counts.max()) // 128) * 128))
    order = np.argsort(idx, kind="stable")
    starts = np.zeros(E + 1, np.int64)
    np.cumsum(counts, out=starts[1:])

    # --- per-core input prep ---
    in_maps = []
    tok_by_core = []
    for c in range(E):
        toks = order[starts[c] : starts[c + 1]]
        tok_by_core.append(toks)
        n = len(toks)
        xg = np.zeros((C, D), np.float32)
        if n:
            xg[:n] = xf[toks]
        # xtr[k, p, t] = xg[t, 128k+p]
        xtr = np.ascontiguousarray(xg.T.reshape(KD, P, C))
        # w1r[i, p, j, c]: j<KD -> x1 block (fc_W[:, 128i+c]); j>=KD -> x2
        w1 = fc_W[c]  # [D, 2F]
        x1 = w1[:, :F].reshape(D, NI, P)       # [D, i, c]
        x2 = w1[:, F:].reshape(D, NI, P)
        w1krc = np.stack([x1, x2], axis=2)     # [D, i, 2, c] -> j pairs
        # want [i, p(of D), j(2*KD: x1 k0..k7 then x2 k0..k7), c]
        w1v = w1krc.reshape(KD, P, NI, 2, P)   # [k, p, i, half, c]
        w1r = np.ascontiguousarray(
            np.transpose(w1v, (2, 1, 3, 0, 4)).reshape(NI, P, 2 * KD, P)
        )
        b1 = fc_b[c]
        b1x1 = b1[:F].reshape(NI, P)           # [i, p]
        b1x2 = b1[F:].reshape(NI, P)
        b1r = np.empty((P, 2 * NI), np.float32)
        b1r[:, 0::2] = b1x1.T
        b1r[:, 1::2] = b1x2.T
        # w2r[m2, p, k2, c] = out_W[c][128k2+p, 128m2+c]
        w2 = out_W[c].reshape(KF, P, M2, P)    # [k2, p, m2, c]
        w2r = np.ascontiguousarray(np.transpose(w2, (2, 1, 0, 3)))
        b2r = np.ascontiguousarray(out_b[c].reshape(M2, P).T)
        # gwr[p, 8k+e] = gate_W[128k+p, e]
        gwr = np.ascontiguousarray(
            np.transpose(gate_W.reshape(KD, P, E), (1, 0, 2)).reshape(P, KD * E)
        )
        gbr = gate_b.reshape(E, 1)
        mask = np.zeros((1, C), np.float32)
        mask[0, :n] = 1.0
        in_maps.append(
            {
                "xtr": _round_fp32r(xtr),
                "w1r": _round_fp32r(w1r),
                "w2r": _round_fp32r(w2r),
                "b1r": b1r,
                "b2r": b2r,
                "gwr": _round_fp32r(gwr),
                "gbr": np.ascontiguousarray(gbr),
                "mkr": mask,
            }
        )

    nc = _build(C)
    res = run_bass_kernel_spmd(
        nc, in_maps, list(range(E)), trace=bool(os.environ.get("KERNEL_TRACE"))
    )
    LAST_RESULTS = res

    out_flat = np.empty((T, D), np.float32)
    scores = np.zeros(E, np.float32)
    for c in range(E):
        ytr = res.results[c]["ytr"]  # [M2, P, C]
        n = len(tok_by_core[c])
        if n:
            yc = ytr.reshape(D, C)[:, :n].T  # [n, D]
            out_flat[tok_by_core[c]] = yc
        scores[c] = res.results[c]["svr"][0, 0]

    usage = scores / (counts.astype(np.float32) + np.float32(1e-8))
    loss = np.float32(np.sum((usage - np.float32(1.0 / E)) ** 2, dtype=np.float32))
    return out_flat.reshape(B, S, D), loss


# revision 7
# speedup vs baseline: 1.2714x; 1.2714x over previous
"""MoE feed-forward (top-1 routing) on 8 Trainium2 NeuronCores.

Strategy (expert-parallel, per sharding hint):
  - Host computes gate logits only to DECIDE routing (argmax), then
    gathers each expert's tokens and ships them (transposed) to the core
    owning that expert, padded to a common capacity C ("all-to-all" done
    host-side since full inputs arrive on host anyway).
  - Each core re-computes the gate softmax max-probability w for its own
    tokens ON DEVICE (gate replicated), runs the expert FFN
    (x @ W1 -> GLU -> @ W2, fp32r matmuls), scales by w, and emits the
    masked sum of w (for the utilization loss).
  - Host scatters rows back and finishes the 8-element loss formula.

Device layout:
  stage 1 keeps features on partitions (H^T = W1.T @ x.T) so the GLU
  pair (x1, x2) lands in adjacent 128-row tiles and the gelu/mul are
  partition-aligned; stage 2 flips to tokens-on-partitions
  (Y = G.T.T @ W2) so the per-token gate weight is a per-partition
  scalar. b2 is folded into stage 2 as a K=1 ones-row accumulation.
"""

import os
import numpy as np

import concourse.bass as bass
import concourse.bacc as bacc
import concourse.mybir as mybir
import concourse.tile as tile
from concourse.bass_utils import run_bass_kernel_spmd

f32 = mybir.dt.float32
f32r = mybir.dt.float32r
AF = mybir.ActivationFunctionType
ALU = mybir.AluOpType

B, S, D, F, E = 4, 2048, 1024, 2048, 8
T = B * S
P = 128
KD = D // P     # 8 k-tiles for matmul1 / gate
KF = F // P     # 16 k-tiles for matmul2
NI = F // P     # 16 feature-block pairs in stage 1
ND = D // 512   # 2 output D-chunks in stage 2

LAST_RESULTS = None


def _round_fp32r(a):
    """Round fp32 array to fp32r (tf32-like: 11-bit mantissa, RNE)."""
    b = np.ascontiguousarray(a, np.float32).view(np.uint32)
    bias = np.uint32(0x7FF) + ((b >> np.uint32(12)) & np.uint32(1))
    return ((b + bias) & np.uint32(0xFFFFF000)).view(np.float32)


def _chunks(C):
    """Split C into column chunks, each a multiple of 128 in [256, 512]."""
    n = -(-C // 512)
    if C // n < 256:
        n = max(1, C // 256)
    base = C // n
    base -= base % 128
    sizes = [base] * n
    rem = C - base * n
    i = 0
    while rem > 0:
        add = min(128, rem)
        sizes[i % n] += add
        rem -= add
        i += 1
    assert sum(sizes) == C and all(s % 128 == 0 and s <= 512 for s in sizes)
    out, c0 = [], 0
    for s in sizes:
        out.append((c0, s))
        c0 += s
    return out


def _build(C):
    nc = bacc.Bacc()
    chunks = _chunks(C)
    NB = C // P

    xtr = nc.declare_dram_parameter("xtr", [KD, P, C], f32r, isOutput=False)
    w1r = nc.declare_dram_parameter("w1r", [NI, P, 2 * KD, P], f32r, isOutput=False)
    w2r = nc.declare_dram_parameter("w2r", [KF, P, D], f32r, isOutput=False)
    b1r = nc.declare_dram_parameter("b1r", [P, 2 * NI], f32, isOutput=False)
    b2r = nc.declare_dram_parameter("b2r", [1, D], f32r, isOutput=False)
    gwr = nc.declare_dram_parameter("gwr", [P, KD * E], f32r, isOutput=False)
    gbr = nc.declare_dram_parameter("gbr", [1, E], f32r, isOutput=False)
    onr = nc.declare_dram_parameter("onr", [1, P], f32r, isOutput=False)
    mkr = nc.declare_dram_parameter("mkr", [P, NB], f32, isOutput=False)
    ytr = nc.declare_dram_parameter("ytr", [C, D], f32, isOutput=True)
    svr = nc.declare_dram_parameter("svr", [1, 1], f32, isOutput=True)

    with tile.TileContext(nc) as tc:
        with (
            tc.tile_pool(name="const", bufs=1) as const,
            tc.tile_pool(name="xt", bufs=1) as xtp,
            tc.tile_pool(name="g", bufs=1) as gp,
            tc.tile_pool(name="w1", bufs=2) as w1p,
            tc.tile_pool(name="w2", bufs=1) as w2p,
            tc.tile_pool(name="ge", bufs=2) as gep,
            tc.tile_pool(name="y", bufs=3) as yp,
            tc.tile_pool(name="gate", bufs=2) as gatep,
            tc.tile_pool(name="psmm", bufs=6, space="PSUM") as psmm,
            tc.tile_pool(name="pssm", bufs=2, space="PSUM") as pssm,
        ):
            # ---- constants / small inputs ----
            b1s = const.tile([P, 2 * NI], f32, tag="b1s")
            nc.sync.dma_start(b1s[:], b1r[:])
            b2row = const.tile([1, D], f32r, tag="b2row")
            nc.sync.dma_start(b2row[:], b2r[:])
            gws = const.tile([P, KD * E], f32r, tag="gws")
            nc.sync.dma_start(gws[:], gwr[:])
            gbs = const.tile([1, E], f32r, tag="gbs")
            nc.sync.dma_start(gbs[:], gbr[:])
            onesr = const.tile([1, P], f32r, tag="onesr")
            nc.sync.dma_start(onesr[:], onr[:])
            maskc = const.tile([P, NB], f32, tag="maskc")
            nc.sync.dma_start(maskc[:], mkr[:])
            ones128 = const.tile([P, 1], f32, tag="ones128")
            nc.vector.memset(ones128[:], 1.0)

            # ---- resident token activations x^T ----
            xt = []
            for k in range(KD):
                t = xtp.tile([P, C], f32r, tag=f"xt{k}", name=f"xt{k}")
                nc.sync.dma_start(t[:], xtr[k])
                xt.append(t)

            # ---- stage 1: H^T pairs -> G^T tiles (features on partitions) ----
            g = [gp.tile([P, C], f32r, tag=f"g{i}", name=f"g{i}") for i in range(NI)]
            for i in range(NI):
                w1t = w1p.tile([P, 2 * KD * P], f32r, tag="w1t")
                nc.sync.dma_start(w1t[:], w1r[i])
                for c0, cl in chunks:
                    psa = psmm.tile([P, 512], f32, tag="ps")
                    for k in range(KD):
                        nc.tensor.matmul(
                            psa[:, :cl],
                            w1t[:, P * k : P * (k + 1)],
                            xt[k][:, c0 : c0 + cl],
                            start=(k == 0),
                            stop=(k == KD - 1),
                        )
                    psb = psmm.tile([P, 512], f32, tag="ps")
                    for k in range(KD):
                        nc.tensor.matmul(
                            psb[:, :cl],
                            w1t[:, P * (KD + k) : P * (KD + k + 1)],
                            xt[k][:, c0 : c0 + cl],
                            start=(k == 0),
                            stop=(k == KD - 1),
                        )
                    get = gep.tile([P, 512], f32, tag="get")
                    nc.scalar.activation(
                        get[:, :cl], psb[:, :cl], AF.Gelu,
                        bias=b1s[:, 2 * i + 1 : 2 * i + 2],
                    )
                    nc.vector.scalar_tensor_tensor(
                        g[i][:, c0 : c0 + cl],
                        psa[:, :cl],
                        b1s[:, 2 * i : 2 * i + 1],
                        get[:, :cl],
                        ALU.add,
                        ALU.mult,
                    )

            # ---- gate: logits [tokens, E] per 128-token block ----
            w_col = const.tile([P, NB], f32, tag="w_col")
            for b in range(NB):
                psl = pssm.tile([P, E], f32, tag="pssmall")
                for k in range(KD):
                    nc.tensor.matmul(
                        psl[:],
                        xt[k][:, P * b : P * (b + 1)],
                        gws[:, E * k : E * (k + 1)],
                        start=(k == 0),
                        stop=False,
                    )
                nc.tensor.matmul(
                    psl[:], onesr[:], gbs[:], start=False, stop=True
                )
                exb = gatep.tile([P, E], f32, tag="exb")
                ssum = gatep.tile([P, 1], f32, tag="ssum")
                nc.scalar.activation(exb[:], psl[:], AF.Exp, accum_out=ssum[:])
                pm = gatep.tile([P, 1], f32, tag="pm")
                nc.vector.reduce_max(pm[:], exb[:], axis=mybir.AxisListType.X)
                rs = gatep.tile([P, 1], f32, tag="rs")
                nc.vector.reciprocal(rs[:], ssum[:])
                nc.vector.tensor_mul(w_col[:, b : b + 1], pm[:], rs[:])

            # masked sum of w -> svr (utilization-loss numerator)
            wm = const.tile([P, NB], f32, tag="wm")
            nc.vector.tensor_mul(wm[:], w_col[:], maskc[:])
            rowsum = const.tile([P, 1], f32, tag="rowsum")
            nc.vector.reduce_sum(rowsum[:], wm[:], axis=mybir.AxisListType.X)
            psc = pssm.tile([1, 1], f32, tag="pssmall")
            nc.tensor.matmul(psc[:], ones128[:], rowsum[:], start=True, stop=True)
            sc = const.tile([1, 1], f32, tag="sc")
            nc.vector.tensor_copy(sc[:], psc[:])
            nc.sync.dma_start(svr[:], sc[:])

            # ---- stage 2: Y[tokens, D] = G.T @ W2 + b2, scaled by w ----
            for dc in range(ND):
                w2h = []
                for k2 in range(KF):
                    t = w2p.tile([P, 512], f32r, tag=f"w2h{k2}", name=f"w2h{k2}")
                    nc.sync.dma_start(t[:], w2r[k2][:, 512 * dc : 512 * (dc + 1)])
                    w2h.append(t)
                for b in range(NB):
                    psy = psmm.tile([P, 512], f32, tag="ps")
                    for k2 in range(KF):
                        nc.tensor.matmul(
                            psy[:],
                            g[k2][:, P * b : P * (b + 1)],
                            w2h[k2][:],
                            start=(k2 == 0),
                            stop=False,
                        )
                    nc.tensor.matmul(
                        psy[:],
                        onesr[:],
                        b2row[:, 512 * dc : 512 * (dc + 1)],
                        start=False,
                        stop=True,
                    )
                    yt = yp.tile([P, 512], f32, tag="yt")
                    nc.vector.tensor_scalar_mul(yt[:], psy[:], w_col[:, b : b + 1])
                    nc.sync.dma_start(
                        ytr[P * b : P * (b + 1), 512 * dc : 512 * (dc + 1)], yt[:]
                    )

    nc.finalize()
    return nc


def kernel(x, gate_W, gate_b, fc_W, fc_b, out_W, out_b):
    global LAST_RESULTS
    x = np.ascontiguousarray(np.asarray(x, dtype=np.float32))
    gate_W = np.asarray(gate_W, dtype=np.float32)
    gate_b = np.asarray(gate_b, dtype=np.float32)
    fc_W = np.asarray(fc_W, dtype=np.float32)
    fc_b = np.asarray(fc_b, dtype=np.float32)
    out_W = np.asarray(out_W, dtype=np.float32)
    out_b = np.asarray(out_b, dtype=np.float32)

    xf = x.reshape(T, D)

    # --- routing decision (host): mirror reference softmax/argmax ---
    logits = xf @ gate_W + gate_b
    lm = logits.max(axis=-1, keepdims=True)
    pe = np.exp(logits - lm)
    probs = pe / pe.sum(axis=-1, keepdims=True)
    idx = np.argmax(probs, axis=-1)
    counts = np.bincount(idx, minlength=E).astype(np.int64)

    C = int(max(512, -(-int(counts.max()) // 128) * 128))
    NB = C // P
    order = np.argsort(idx, kind="stable")
    starts = np.zeros(E + 1, np.int64)
    np.cumsum(counts, out=starts[1:])

    # gate weights in device layout: gwr[p, 8k+e] = gate_W[128k+p, e]
    gwr = _round_fp32r(
        np.transpose(gate_W.reshape(KD, P, E), (1, 0, 2)).reshape(P, KD * E)
    )
    gbr = _round_fp32r(gate_b.reshape(1, E))
    onr = np.ones((1, P), np.float32)

    in_maps = []
    tok_by_core = []
    for c in range(E):
        toks = order[starts[c] : starts[c + 1]]
        tok_by_core.append(toks)
        n = len(toks)
        xg = np.zeros((C, D), np.float32)
        if n:
            xg[:n] = xf[toks]
        # xtr[k, p, t] = xg[t, 128k+p]
        xtr = np.ascontiguousarray(xg.T.reshape(KD, P, C))
        # w1r[i, p, j, c]: j<KD -> x1 block (fc_W[:, 128i+c]); j>=KD -> x2
        w1 = fc_W[c]  # [D, 2F]
        x1 = w1[:, :F].reshape(D, NI, P)       # [D, i, c]
        x2 = w1[:, F:].reshape(D, NI, P)
        w1krc = np.stack([x1, x2], axis=2)     # [D, i, 2, c]
        w1v = w1krc.reshape(KD, P, NI, 2, P)   # [k, p, i, half, c]
        w1r = np.ascontiguousarray(
            np.transpose(w1v, (2, 1, 3, 0, 4)).reshape(NI, P, 2 * KD, P)
        )
        b1 = fc_b[c]
        b1x1 = b1[:F].reshape(NI, P)           # [i, p]
        b1x2 = b1[F:].reshape(NI, P)
        b1r = np.empty((P, 2 * NI), np.float32)
        b1r[:, 0::2] = b1x1.T
        b1r[:, 1::2] = b1x2.T
        # w2r[k2, p, d] = out_W[c][128k2+p, d]  (natural layout)
        w2r = _round_fp32r(out_W[c]).reshape(KF, P, D)
        b2rr = _round_fp32r(out_b[c].reshape(1, D))
        mask = np.zeros((P, NB), np.float32)
        if n:
            tid = np.arange(P)[:, None] + P * np.arange(NB)[None, :]
            mask[tid < n] = 1.0
        in_maps.append(
            {
                "xtr": _round_fp32r(xtr),
                "w1r": _round_fp32r(w1r),
                "w2r": np.ascontiguousarray(w2r),
                "b1r": b1r,
                "b2r": b2rr,
                "gwr": gwr,
                "gbr": gbr,
                "onr": onr,
                "mkr": mask,
            }
        )

    nc = _build(C)
    res = run_bass_kernel_spmd(
        nc, in_maps, list(range(E)), trace=bool(os.environ.get("KERNEL_TRACE"))
    )
    LAST_RESULTS = res

    out_flat = np.empty((T, D), np.float32)
    scores = np.zeros(E, np.float32)
    for c in range(E):
        yc = res.results[c]["ytr"]  # [C, D]
        n = len(tok_by_core[c])
        if n:
            out_flat[tok_by_core[c]] = yc[:n]
        scores[c] = res.results[c]["svr"][0, 0]

    usage = scores / (counts.astype(np.float32) + np.float32(1e-8))
    loss = np.float32(np.sum((usage - np.float32(1.0 / E)) ** 2, dtype=np.float32))
    return out_flat.reshape(B, S, D), loss


# revision 8
# speedup vs baseline: 1.3085x; 1.0292x over previous
"""MoE feed-forward (top-1 routing) on 8 Trainium2 NeuronCores.

Strategy (expert-parallel, per sharding hint):
  - Host computes gate logits only to DECIDE routing (argmax), then
    gathers each expert's tokens and ships them (transposed) to the core
    owning that expert, padded to a common capacity C ("all-to-all" done
    host-side since full inputs arrive on host anyway).
  - Each core re-computes the gate softmax max-probability w for its own
    tokens ON DEVICE (gate replicated), runs the expert FFN
    (x @ W1 -> GLU -> @ W2, fp32r matmuls), scales by w, and emits the
    masked sum of w (for the utilization loss).
  - Host scatters rows back and finishes the 8-element loss formula.

Device layout:
  stage 1 keeps features on partitions (H^T = W1.T @ x.T) so the GLU
  pair (x1, x2) lands in adjacent 128-row tiles and the gelu/mul are
  partition-aligned; stage 2 flips to tokens-on-partitions
  (Y = G.T.T @ W2) so the per-token gate weight is a per-partition
  scalar. b2 is folded into stage 2 as a K=1 ones-row accumulation.
"""

import os
import numpy as np

import concourse.bass as bass
import concourse.bacc as bacc
import concourse.mybir as mybir
import concourse.tile as tile
from concourse.bass_utils import run_bass_kernel_spmd

f32 = mybir.dt.float32
f32r = mybir.dt.float32r
AF = mybir.ActivationFunctionType
ALU = mybir.AluOpType

B, S, D, F, E = 4, 2048, 1024, 2048, 8
T = B * S
P = 128
KD = D // P     # 8 k-tiles for matmul1 / gate
KF = F // P     # 16 k-tiles for matmul2
NI = F // P     # 16 feature-block pairs in stage 1
ND = D // 512   # 2 output D-chunks in stage 2

LAST_RESULTS = None


def _round_fp32r(a):
    """Round fp32 array to fp32r (tf32-like: 11-bit mantissa, RNE)."""
    b = np.ascontiguousarray(a, np.float32).view(np.uint32)
    bias = np.uint32(0x7FF) + ((b >> np.uint32(12)) & np.uint32(1))
    return ((b + bias) & np.uint32(0xFFFFF000)).view(np.float32)


def _chunks(C):
    """Split C into column chunks, each a multiple of 128 in [256, 512]."""
    n = -(-C // 512)
    if C // n < 256:
        n = max(1, C // 256)
    base = C // n
    base -= base % 128
    sizes = [base] * n
    rem = C - base * n
    i = 0
    while rem > 0:
        add = min(128, rem)
        sizes[i % n] += add
        rem -= add
        i += 1
    assert sum(sizes) == C and all(s % 128 == 0 and s <= 512 for s in sizes)
    out, c0 = [], 0
    for s in sizes:
        out.append((c0, s))
        c0 += s
    return out


def _build(C):
    nc = bacc.Bacc()
    chunks = _chunks(C)
    NB = C // P

    xtr = nc.declare_dram_parameter("xtr", [KD, P, C], f32r, isOutput=False)
    w1r = nc.declare_dram_parameter("w1r", [NI, P, 2 * KD, P], f32r, isOutput=False)
    w2r = nc.declare_dram_parameter("w2r", [KF, P, D], f32r, isOutput=False)
    b1r = nc.declare_dram_parameter("b1r", [P, 2 * NI], f32, isOutput=False)
    b2r = nc.declare_dram_parameter("b2r", [1, D], f32r, isOutput=False)
    gwr = nc.declare_dram_parameter("gwr", [P, KD * E], f32r, isOutput=False)
    gbr = nc.declare_dram_parameter("gbr", [1, E], f32r, isOutput=False)
    onr = nc.declare_dram_parameter("onr", [1, P], f32r, isOutput=False)
    mkr = nc.declare_dram_parameter("mkr", [P, NB], f32, isOutput=False)
    ytr = nc.declare_dram_parameter("ytr", [C, D], f32, isOutput=True)
    svr = nc.declare_dram_parameter("svr", [1, 1], f32, isOutput=True)

    with tile.TileContext(nc) as tc:
        with (
            tc.tile_pool(name="const", bufs=1) as const,
            tc.tile_pool(name="xt", bufs=1) as xtp,
            tc.tile_pool(name="g", bufs=1) as gp,
            tc.tile_pool(name="w1", bufs=2) as w1p,
            tc.tile_pool(name="w2", bufs=1) as w2p,
            tc.tile_pool(name="ge", bufs=2) as gep,
            tc.tile_pool(name="y", bufs=3) as yp,
            tc.tile_pool(name="gate", bufs=2) as gatep,
            tc.tile_pool(name="psmm", bufs=6, space="PSUM") as psmm,
            tc.tile_pool(name="pssm", bufs=2, space="PSUM") as pssm,
        ):
            # ---- constants / small inputs ----
            b1s = const.tile([P, 2 * NI], f32, tag="b1s")
            nc.sync.dma_start(b1s[:], b1r[:])
            b2row = const.tile([1, D], f32r, tag="b2row")
            nc.sync.dma_start(b2row[:], b2r[:])
            gws = const.tile([P, KD * E], f32r, tag="gws")
            nc.sync.dma_start(gws[:], gwr[:])
            gbs = const.tile([1, E], f32r, tag="gbs")
            nc.sync.dma_start(gbs[:], gbr[:])
            onesr = const.tile([1, P], f32r, tag="onesr")
            nc.sync.dma_start(onesr[:], onr[:])
            maskc = const.tile([P, NB], f32, tag="maskc")
            nc.sync.dma_start(maskc[:], mkr[:])
            ones128 = const.tile([P, 1], f32, tag="ones128")
            nc.vector.memset(ones128[:], 1.0)

            # ---- resident token activations x^T ----
            # Loaded per column-chunk so stage-1 compute on chunk 0 can
            # start after ~1/3 of the x traffic; the first two W1 strips
            # are prefetched between chunk slices to avoid PE starvation.
            xt = [
                xtp.tile([P, C], f32r, tag=f"xt{k}", name=f"xt{k}")
                for k in range(KD)
            ]
            c0_, cl_ = chunks[0]
            for k in range(KD):
                nc.sync.dma_start(xt[k][:, c0_ : c0_ + cl_], xtr[k][:, c0_ : c0_ + cl_])
            w1pre = []
            for i in range(min(2, NI)):
                t = w1p.tile([P, 2 * KD * P], f32r, tag="w1t", name=f"w1pre{i}")
                nc.sync.dma_start(t[:], w1r[i])
                w1pre.append(t)
            for c0_, cl_ in chunks[1:]:
                for k in range(KD):
                    nc.sync.dma_start(
                        xt[k][:, c0_ : c0_ + cl_], xtr[k][:, c0_ : c0_ + cl_]
                    )

            # ---- stage 1: H^T pairs -> G^T tiles (features on partitions) ----
            g = [gp.tile([P, C], f32r, tag=f"g{i}", name=f"g{i}") for i in range(NI)]
            for i in range(NI):
                if i < len(w1pre):
                    w1t = w1pre[i]
                else:
                    w1t = w1p.tile([P, 2 * KD * P], f32r, tag="w1t")
                    nc.sync.dma_start(w1t[:], w1r[i])
                for c0, cl in chunks:
                    psa = psmm.tile([P, 512], f32, tag="ps")
                    for k in range(KD):
                        nc.tensor.matmul(
                            psa[:, :cl],
                            w1t[:, P * k : P * (k + 1)],
                            xt[k][:, c0 : c0 + cl],
                            start=(k == 0),
                            stop=(k == KD - 1),
                        )
                    psb = psmm.tile([P, 512], f32, tag="ps")
                    for k in range(KD):
                        nc.tensor.matmul(
                            psb[:, :cl],
                            w1t[:, P * (KD + k) : P * (KD + k + 1)],
                            xt[k][:, c0 : c0 + cl],
                            start=(k == 0),
                            stop=(k == KD - 1),
                        )
                    get = gep.tile([P, 512], f32, tag="get")
                    nc.scalar.activation(
                        get[:, :cl], psb[:, :cl], AF.Gelu,
                        bias=b1s[:, 2 * i + 1 : 2 * i + 2],
                    )
                    nc.vector.scalar_tensor_tensor(
                        g[i][:, c0 : c0 + cl],
                        psa[:, :cl],
                        b1s[:, 2 * i : 2 * i + 1],
                        get[:, :cl],
                        ALU.add,
                        ALU.mult,
                    )

            # ---- gate: logits [tokens, E] per 128-token block ----
            w_col = const.tile([P, NB], f32, tag="w_col")
            for b in range(NB):
                psl = pssm.tile([P, E], f32, tag="pssmall")
                for k in range(KD):
                    nc.tensor.matmul(
                        psl[:],
                        xt[k][:, P * b : P * (b + 1)],
                        gws[:, E * k : E * (k + 1)],
                        start=(k == 0),
                        stop=False,
                    )
                nc.tensor.matmul(
                    psl[:], onesr[:], gbs[:], start=False, stop=True
                )
                exb = gatep.tile([P, E], f32, tag="exb")
                ssum = gatep.tile([P, 1], f32, tag="ssum")
                nc.scalar.activation(exb[:], psl[:], AF.Exp, accum_out=ssum[:])
                pm = gatep.tile([P, 1], f32, tag="pm")
                nc.vector.reduce_max(pm[:], exb[:], axis=mybir.AxisListType.X)
                rs = gatep.tile([P, 1], f32, tag="rs")
                nc.vector.reciprocal(rs[:], ssum[:])
                nc.vector.tensor_mul(w_col[:, b : b + 1], pm[:], rs[:])

            # masked sum of w -> svr (utilization-loss numerator)
            wm = const.tile([P, NB], f32, tag="wm")
            nc.vector.tensor_mul(wm[:], w_col[:], maskc[:])
            rowsum = const.tile([P, 1], f32, tag="rowsum")
            nc.vector.reduce_sum(rowsum[:], wm[:], axis=mybir.AxisListType.X)
            psc = pssm.tile([1, 1], f32, tag="pssmall")
            nc.tensor.matmul(psc[:], ones128[:], rowsum[:], start=True, stop=True)
            sc = const.tile([1, 1], f32, tag="sc")
            nc.vector.tensor_copy(sc[:], psc[:])
            nc.sync.dma_start(svr[:], sc[:])

            # ---- stage 2: Y[tokens, D] = G.T @ W2 + b2, scaled by w ----
            for dc in range(ND):
                w2h = []
                for k2 in range(KF):
                    t = w2p.tile([P, 512], f32r, tag=f"w2h{k2}", name=f"w2h{k2}")
                    nc.sync.dma_start(t[:], w2r[k2][:, 512 * dc : 512 * (dc + 1)])
                    w2h.append(t)
                for b in range(NB):
                    psy = psmm.tile([P, 512], f32, tag="ps")
                    for k2 in range(KF):
                        nc.tensor.matmul(
                            psy[:],
                            g[k2][:, P * b : P * (b + 1)],
                            w2h[k2][:],
                            start=(k2 == 0),
                            stop=False,
                        )
                    nc.tensor.matmul(
                        psy[:],
                        onesr[:],
                        b2row[:, 512 * dc : 512 * (dc + 1)],
                        start=False,
                        stop=True,
                    )
                    yt = yp.tile([P, 512], f32, tag="yt")
                    nc.vector.tensor_scalar_mul(yt[:], psy[:], w_col[:, b : b + 1])
                    nc.sync.dma_start(
                        ytr[P * b : P * (b + 1), 512 * dc : 512 * (dc + 1)], yt[:]
                    )

    nc.finalize()
    return nc


def kernel(x, gate_W, gate_b, fc_W, fc_b, out_W, out_b):
    global LAST_RESULTS
    x = np.ascontiguousarray(np.asarray(x, dtype=np.float32))
    gate_W = np.asarray(gate_W, dtype=np.float32)
    gate_b = np.asarray(gate_b, dtype=np.float32)
    fc_W = np.asarray(fc_W, dtype=np.float32)
    fc_b = np.asarray(fc_b, dtype=np.float32)
    out_W = np.asarray(out_W, dtype=np.float32)
    out_b = np.asarray(out_b, dtype=np.float32)

    xf = x.reshape(T, D)

    # --- routing decision (host): mirror reference softmax/argmax ---
    logits = xf @ gate_W + gate_b
    lm = logits.max(axis=-1, keepdims=True)
    pe = np.exp(logits - lm)
    probs = pe / pe.sum(axis=-1, keepdims=True)
    idx = np.argmax(probs, axis=-1)
    counts = np.bincount(idx, minlength=E).astype(np.int64)

    C = int(max(512, -(-int(counts.max()) // 128) * 128))
    NB = C // P
    order = np.argsort(idx, kind="stable")
    starts = np.zeros(E + 1, np.int64)
    np.cumsum(counts, out=starts[1:])

    # gate weights in device layout: gwr[p, 8k+e] = gate_W[128k+p, e]
    gwr = _round_fp32r(
        np.transpose(gate_W.reshape(KD, P, E), (1, 0, 2)).reshape(P, KD * E)
    )
    gbr = _round_fp32r(gate_b.reshape(1, E))
    onr = np.ones((1, P), np.float32)

    in_maps = []
    tok_by_core = []
    for c in range(E):
        toks = order[starts[c] : starts[c + 1]]
        tok_by_core.append(toks)
        n = len(toks)
        xg = np.zeros((C, D), np.float32)
        if n:
            xg[:n] = xf[toks]
        # xtr[k, p, t] = xg[t, 128k+p]
        xtr = np.ascontiguousarray(xg.T.reshape(KD, P, C))
        # w1r[i, p, j, c]: j<KD -> x1 block (fc_W[:, 128i+c]); j>=KD -> x2
        w1 = fc_W[c]  # [D, 2F]
        x1 = w1[:, :F].reshape(D, NI, P)       # [D, i, c]
        x2 = w1[:, F:].reshape(D, NI, P)
        w1krc = np.stack([x1, x2], axis=2)     # [D, i, 2, c]
        w1v = w1krc.reshape(KD, P, NI, 2, P)   # [k, p, i, half, c]
        w1r = np.ascontiguousarray(
            np.transpose(w1v, (2, 1, 3, 0, 4)).reshape(NI, P, 2 * KD, P)
        )
        b1 = fc_b[c]
        b1x1 = b1[:F].reshape(NI, P)           # [i, p]
        b1x2 = b1[F:].reshape(NI, P)
        b1r = np.empty((P, 2 * NI), np.float32)
        b1r[:, 0::2] = b1x1.T
        b1r[:, 1::2] = b1x2.T
        # w2r[k2, p, d] = out_W[c][128k2+p, d]  (natural layout)
        w2r = _round_fp32r(out_W[c]).reshape(KF, P, D)
        b2rr = _round_fp32r(out_b[c].reshape(1, D))
        mask = np.zeros((P, NB), np.float32)
        if n:
            tid = np.arange(P)[:, None] + P * np.arange(NB)[None, :]
            mask[tid < n] = 1.0
        in_maps.append(
            {
                "xtr": _round_fp32r(xtr),
                "w1r": _round_fp32r(w1r),
                "w2r": np.ascontiguousarray(w2r),
                "b1r": b1r,
                "b2r": b2rr,
                "gwr": gwr,
                "gbr": gbr,
                "onr": onr,
                "mkr": mask,
            }
        )

    nc = _build(C)
    res = run_bass_kernel_spmd(
        nc, in_maps, list(range(E)), trace=bool(os.environ.get("KERNEL_TRACE"))
    )
    LAST_RESULTS = res

    out_flat = np.empty((T, D), np.float32)
    scores = np.zeros(E, np.float32)
    for c in range(E):
        yc = res.results[c]["ytr"]  # [C, D]
        n = len(tok_by_core[c])
        if n:
            out_flat[tok_by_core[c]] = yc[:n]
        scores[c] = res.results[c]["svr"][0, 0]

    usage = scores / (counts.astype(np.float32) + np.float32(1e-8))
    loss = np.float32(np.sum((usage - np.float32(1.0 / E)) ** 2, dtype=np.float32))
    return out_flat.reshape(B, S, D), loss


# revision 9
# speedup vs baseline: 1.3342x; 1.0196x over previous
"""MoE feed-forward (top-1 routing) on 8 Trainium2 NeuronCores.

Strategy (expert-parallel, per sharding hint):
  - Host computes gate logits only to DECIDE routing (argmax), then
    gathers each expert's tokens and ships them (transposed) to the core
    owning that expert, padded to a common capacity C ("all-to-all" done
    host-side since full inputs arrive on host anyway).
  - Each core re-computes the gate softmax max-probability w for its own
    tokens ON DEVICE (gate replicated), runs the expert FFN
    (x @ W1 -> GLU -> @ W2, fp32r matmuls), scales by w, and emits the
    masked sum of w (for the utilization loss).
  - Host scatters rows back and finishes the 8-element loss formula.

Device layout:
  stage 1 keeps features on partitions (H^T = W1.T @ x.T) so the GLU
  pair (x1, x2) lands in adjacent 128-row tiles and the gelu/mul are
  partition-aligned; stage 2 flips to tokens-on-partitions
  (Y = G.T.T @ W2) so the per-token gate weight is a per-partition
  scalar. b2 is folded into stage 2 as a K=1 ones-row accumulation.
"""

import os
import numpy as np

import concourse.bass as bass
import concourse.bacc as bacc
import concourse.mybir as mybir
import concourse.tile as tile
from concourse.bass_utils import run_bass_kernel_spmd

f32 = mybir.dt.float32
f32r = mybir.dt.float32r
AF = mybir.ActivationFunctionType
ALU = mybir.AluOpType

B, S, D, F, E = 4, 2048, 1024, 2048, 8
T = B * S
P = 128
KD = D // P     # 8 k-tiles for matmul1 / gate
KF = F // P     # 16 k-tiles for matmul2
NI = F // P     # 16 feature-block pairs in stage 1
ND = D // 512   # 2 output D-chunks in stage 2

LAST_RESULTS = None


def _round_fp32r(a):
    """Round fp32 array to fp32r (tf32-like: 11-bit mantissa, RNE)."""
    b = np.ascontiguousarray(a, np.float32).view(np.uint32)
    bias = np.uint32(0x7FF) + ((b >> np.uint32(12)) & np.uint32(1))
    return ((b + bias) & np.uint32(0xFFFFF000)).view(np.float32)


def _chunks(C):
    """Split C into column chunks, each a multiple of 128 in [256, 512]."""
    n = -(-C // 512)
    if C // n < 256:
        n = max(1, C // 256)
    base = C // n
    base -= base % 128
    sizes = [base] * n
    rem = C - base * n
    i = 0
    while rem > 0:
        add = min(128, rem)
        sizes[i % n] += add
        rem -= add
        i += 1
    assert sum(sizes) == C and all(s % 128 == 0 and s <= 512 for s in sizes)
    sizes.sort()  # smallest first: shortens the startup critical path
    out, c0 = [], 0
    for s in sizes:
        out.append((c0, s))
        c0 += s
    return out


def _build(C):
    nc = bacc.Bacc()
    chunks = _chunks(C)
    NB = C // P

    xtr = nc.declare_dram_parameter("xtr", [KD, P, C], f32r, isOutput=False)
    w1r = nc.declare_dram_parameter("w1r", [NI, P, 2 * KD, P], f32r, isOutput=False)
    w2r = nc.declare_dram_parameter("w2r", [KF, P, D], f32r, isOutput=False)
    b1r = nc.declare_dram_parameter("b1r", [P, 2 * NI], f32, isOutput=False)
    b2r = nc.declare_dram_parameter("b2r", [1, D], f32r, isOutput=False)
    gwr = nc.declare_dram_parameter("gwr", [P, KD * E], f32r, isOutput=False)
    gbr = nc.declare_dram_parameter("gbr", [1, E], f32r, isOutput=False)
    onr = nc.declare_dram_parameter("onr", [1, P], f32r, isOutput=False)
    mkr = nc.declare_dram_parameter("mkr", [P, NB], f32, isOutput=False)
    ytr = nc.declare_dram_parameter("ytr", [C, D], f32, isOutput=True)
    svr = nc.declare_dram_parameter("svr", [1, 1], f32, isOutput=True)

    with tile.TileContext(nc) as tc:
        with (
            tc.tile_pool(name="const", bufs=1) as const,
            tc.tile_pool(name="xt", bufs=1) as xtp,
            tc.tile_pool(name="g", bufs=1) as gp,
            tc.tile_pool(name="w1", bufs=2) as w1p,
            tc.tile_pool(name="w2", bufs=1) as w2p,
            tc.tile_pool(name="ge", bufs=2) as gep,
            tc.tile_pool(name="y", bufs=3) as yp,
            tc.tile_pool(name="gate", bufs=2) as gatep,
            tc.tile_pool(name="psmm", bufs=6, space="PSUM") as psmm,
            tc.tile_pool(name="pssm", bufs=2, space="PSUM") as pssm,
        ):
            # ---- constants / small inputs ----
            b1s = const.tile([P, 2 * NI], f32, tag="b1s")
            nc.sync.dma_start(b1s[:], b1r[:])
            b2row = const.tile([1, D], f32r, tag="b2row")
            nc.sync.dma_start(b2row[:], b2r[:])
            gws = const.tile([P, KD * E], f32r, tag="gws")
            nc.sync.dma_start(gws[:], gwr[:])
            gbs = const.tile([1, E], f32r, tag="gbs")
            nc.sync.dma_start(gbs[:], gbr[:])
            onesr = const.tile([1, P], f32r, tag="onesr")
            nc.sync.dma_start(onesr[:], onr[:])
            maskc = const.tile([P, NB], f32, tag="maskc")
            nc.sync.dma_start(maskc[:], mkr[:])
            ones128 = const.tile([P, 1], f32, tag="ones128")
            nc.vector.memset(ones128[:], 1.0)

            # ---- resident token activations x^T ----
            # Loaded per column-chunk so stage-1 compute on chunk 0 can
            # start after ~1/3 of the x traffic; the first two W1 strips
            # are prefetched between chunk slices to avoid PE starvation.
            xt = [
                xtp.tile([P, C], f32r, tag=f"xt{k}", name=f"xt{k}")
                for k in range(KD)
            ]
            c0_, cl_ = chunks[0]
            for k in range(KD):
                nc.sync.dma_start(xt[k][:, c0_ : c0_ + cl_], xtr[k][:, c0_ : c0_ + cl_])
            w1pre = []
            for i in range(min(2, NI)):
                t = w1p.tile([P, 2 * KD * P], f32r, tag="w1t", name=f"w1pre{i}")
                nc.sync.dma_start(t[:], w1r[i])
                w1pre.append(t)
            for c0_, cl_ in chunks[1:]:
                for k in range(KD):
                    nc.sync.dma_start(
                        xt[k][:, c0_ : c0_ + cl_], xtr[k][:, c0_ : c0_ + cl_]
                    )

            # ---- stage 1: H^T pairs -> G^T tiles (features on partitions) ----
            g = [gp.tile([P, C], f32r, tag=f"g{i}", name=f"g{i}") for i in range(NI)]
            for i in range(NI):
                if i < len(w1pre):
                    w1t = w1pre[i]
                else:
                    w1t = w1p.tile([P, 2 * KD * P], f32r, tag="w1t")
                    nc.sync.dma_start(w1t[:], w1r[i])
                for c0, cl in chunks:
                    psa = psmm.tile([P, 512], f32, tag="ps")
                    for k in range(KD):
                        nc.tensor.matmul(
                            psa[:, :cl],
                            w1t[:, P * k : P * (k + 1)],
                            xt[k][:, c0 : c0 + cl],
                            start=(k == 0),
                            stop=(k == KD - 1),
                        )
                    psb = psmm.tile([P, 512], f32, tag="ps")
                    for k in range(KD):
                        nc.tensor.matmul(
                            psb[:, :cl],
                            w1t[:, P * (KD + k) : P * (KD + k + 1)],
                            xt[k][:, c0 : c0 + cl],
                            start=(k == 0),
                            stop=(k == KD - 1),
                        )
                    get = gep.tile([P, 512], f32, tag="get")
                    nc.scalar.activation(
                        get[:, :cl], psb[:, :cl], AF.Gelu,
                        bias=b1s[:, 2 * i + 1 : 2 * i + 2],
                    )
                    nc.vector.scalar_tensor_tensor(
                        g[i][:, c0 : c0 + cl],
                        psa[:, :cl],
                        b1s[:, 2 * i : 2 * i + 1],
                        get[:, :cl],
                        ALU.add,
                        ALU.mult,
                    )

            # ---- prefetch all W2 strips (DMAs overlap stage-1 compute) ----
            w2pre = []
            for dc in range(ND):
                row = []
                for k2 in range(KF):
                    t = w2p.tile(
                        [P, 512], f32r, tag=f"w2h{dc}_{k2}", name=f"w2h{dc}_{k2}"
                    )
                    nc.sync.dma_start(t[:], w2r[k2][:, 512 * dc : 512 * (dc + 1)])
                    row.append(t)
                w2pre.append(row)

            # ---- gate: logits [tokens, E] per 128-token block ----
            w_col = const.tile([P, NB], f32, tag="w_col")
            for b in range(NB):
                psl = pssm.tile([P, E], f32, tag="pssmall")
                for k in range(KD):
                    nc.tensor.matmul(
                        psl[:],
                        xt[k][:, P * b : P * (b + 1)],
                        gws[:, E * k : E * (k + 1)],
                        start=(k == 0),
                        stop=False,
                    )
                nc.tensor.matmul(
                    psl[:], onesr[:], gbs[:], start=False, stop=True
                )
                exb = gatep.tile([P, E], f32, tag="exb")
                ssum = gatep.tile([P, 1], f32, tag="ssum")
                nc.scalar.activation(exb[:], psl[:], AF.Exp, accum_out=ssum[:])
                pm = gatep.tile([P, 1], f32, tag="pm")
                nc.vector.reduce_max(pm[:], exb[:], axis=mybir.AxisListType.X)
                rs = gatep.tile([P, 1], f32, tag="rs")
                nc.vector.reciprocal(rs[:], ssum[:])
                nc.vector.tensor_mul(w_col[:, b : b + 1], pm[:], rs[:])


            # ---- stage 2: Y[tokens, D] = G.T @ W2 + b2, scaled by w ----
            for dc in range(ND):
                w2h = w2pre[dc]
                for b in range(NB):
                    psy = psmm.tile([P, 512], f32, tag="ps")
                    for k2 in range(KF):
                        nc.tensor.matmul(
                            psy[:],
                            g[k2][:, P * b : P * (b + 1)],
                            w2h[k2][:],
                            start=(k2 == 0),
                            stop=False,
                        )
                    nc.tensor.matmul(
                        psy[:],
                        onesr[:],
                        b2row[:, 512 * dc : 512 * (dc + 1)],
                        start=False,
                        stop=True,
                    )
                    yt = yp.tile([P, 512], f32, tag="yt")
                    nc.vector.tensor_scalar_mul(yt[:], psy[:], w_col[:, b : b + 1])
                    nc.sync.dma_start(
                        ytr[P * b : P * (b + 1), 512 * dc : 512 * (dc + 1)], yt[:]
                    )

            # masked sum of w -> svr (utilization-loss numerator)
            wm = const.tile([P, NB], f32, tag="wm")
            nc.vector.tensor_mul(wm[:], w_col[:], maskc[:])
            rowsum = const.tile([P, 1], f32, tag="rowsum")
            nc.vector.reduce_sum(rowsum[:], wm[:], axis=mybir.AxisListType.X)
            psc = pssm.tile([1, 1], f32, tag="pssmall")
            nc.tensor.matmul(psc[:], ones128[:], rowsum[:], start=True, stop=True)
            sc = const.tile([1, 1], f32, tag="sc")
            nc.vector.tensor_copy(sc[:], psc[:])
            nc.sync.dma_start(svr[:], sc[:])

    nc.finalize()
    return nc


def kernel(x, gate_W, gate_b, fc_W, fc_b, out_W, out_b):
    global LAST_RESULTS
    x = np.ascontiguousarray(np.asarray(x, dtype=np.float32))
    gate_W = np.asarray(gate_W, dtype=np.float32)
    gate_b = np.asarray(gate_b, dtype=np.float32)
    fc_W = np.asarray(fc_W, dtype=np.float32)
    fc_b = np.asarray(fc_b, dtype=np.float32)
    out_W = np.asarray(out_W, dtype=np.float32)
    out_b = np.asarray(out_b, dtype=np.float32)

    xf = x.reshape(T, D)

    # --- routing decision (host): mirror reference softmax/argmax ---
    logits = xf @ gate_W + gate_b
    lm = logits.max(axis=-1, keepdims=True)
    pe = np.exp(logits - lm)
    probs = pe / pe.sum(axis=-1, keepdims=True)
    idx = np.argmax(probs, axis=-1)
    counts = np.bincount(idx, minlength=E).astype(np.int64)

    C = int(max(512, -(-int(counts.max()) // 128) * 128))
    NB = C // P
    order = np.argsort(idx, kind="stable")
    starts = np.zeros(E + 1, np.int64)
    np.cumsum(counts, out=starts[1:])

    # gate weights in device layout: gwr[p, 8k+e] = gate_W[128k+p, e]
    gwr = _round_fp32r(
        np.transpose(gate_W.reshape(KD, P, E), (1, 0, 2)).reshape(P, KD * E)
    )
    gbr = _round_fp32r(gate_b.reshape(1, E))
    onr = np.ones((1, P), np.float32)

    in_maps = []
    tok_by_core = []
    for c in range(E):
        toks = order[starts[c] : starts[c + 1]]
        tok_by_core.append(toks)
        n = len(toks)
        xg = np.zeros((C, D), np.float32)
        if n:
            xg[:n] = xf[toks]
        # xtr[k, p, t] = xg[t, 128k+p]
        xtr = np.ascontiguousarray(xg.T.reshape(KD, P, C))
        # w1r[i, p, j, c]: j<KD -> x1 block (fc_W[:, 128i+c]); j>=KD -> x2
        w1 = fc_W[c]  # [D, 2F]
        x1 = w1[:, :F].reshape(D, NI, P)       # [D, i, c]
        x2 = w1[:, F:].reshape(D, NI, P)
        w1krc = np.stack([x1, x2], axis=2)     # [D, i, 2, c]
        w1v = w1krc.reshape(KD, P, NI, 2, P)   # [k, p, i, half, c]
        w1r = np.ascontiguousarray(
            np.transpose(w1v, (2, 1, 3, 0, 4)).reshape(NI, P, 2 * KD, P)
        )
        b1 = fc_b[c]
        b1x1 = b1[:F].reshape(NI, P)           # [i, p]
        b1x2 = b1[F:].reshape(NI, P)
        b1r = np.empty((P, 2 * NI), np.float32)
        b1r[:, 0::2] = b1x1.T
        b1r[:, 1::2] = b1x2.T
        # w2r[k2, p, d] = out_W[c][128k2+p, d]  (natural layout)
        w2r = _round_fp32r(out_W[c]).reshape(KF, P, D)
        b2rr = _round_fp32r(out_b[c].reshape(1, D))
        mask = np.zeros((P, NB), np.float32)
        if n:
            tid = np.arange(P)[:, None] + P * np.arange(NB)[None, :]
            mask[tid < n] = 1.0
        in_maps.append(
            {
                "xtr": _round_fp32r(xtr),
                "w1r": _round_fp32r(w1r),
                "w2r": np.ascontiguousarray(w2r),
                "b1r": b1r,
                "b2r": b2rr,
                "gwr": gwr,
                "gbr": gbr,
                "onr": onr,
                "mkr": mask,
            }
        )

    nc = _build(C)
    res = run_bass_kernel_spmd(
        nc, in_maps, list(range(E)), trace=bool(os.environ.get("KERNEL_TRACE"))
    )
    LAST_RESULTS = res

    out_flat = np.empty((T, D), np.float32)
    scores = np.zeros(E, np.float32)
    for c in range(E):
        yc = res.results[c]["ytr"]  # [C, D]
        n = len(tok_by_core[c])
        if n:
            out_flat[tok_by_core[c]] = yc[:n]
        scores[c] = res.results[c]["svr"][0, 0]

    usage = scores / (counts.astype(np.float32) + np.float32(1e-8))
    loss = np.float32(np.sum((usage - np.float32(1.0 / E)) ** 2, dtype=np.float32))
    return out_flat.reshape(B, S, D), loss
